# revision 1
# baseline (speedup 1.0000x reference)
"""Trainium2 Bass kernel for nn_Encoder_base (5x ChebConv GNN + pool + MLP).

Distribution over 8 NeuronCores:
  - level-0 ChebConv props: edge-sharded by destination (all 32 batches,
    96 = 32*3 features), selection-matmul scatter + 2 DRAM AllGathers
  - levels 1-3: batch-sharded (4 batches/core, 128 = 4*32 features);
    level-1 props sparse (indirect-DMA row gathers + selection matmuls),
    levels 2-3 dense-S matmuls
  - channel mixes as block-diagonal matmuls in feature-major layout
  - MLP: output-feature sharded (each core owns 512 cols of W6/W7/W8 and
    512 rows of W9), BatchNorm is local per feature; activations AllGathered
"""
import numpy as np
import concourse.bass as bass
import concourse.bacc as bacc
import concourse.tile as tile
from concourse import mybir, bass_utils
from concourse.masks import make_identity

F32 = mybir.dt.float32
I32 = mybir.dt.int32
I16 = mybir.dt.int16
AF = mybir.ActivationFunctionType
ALU = mybir.AluOpType
AX = mybir.AxisListType
RG = [list(range(8))]
NCORES = 8
N0, N1, N2, N3 = 16384, 4096, 1024, 128
EPS = 1e-5

_CACHE = {}


# ---------------------------------------------------------------- host prep
def _prep_prop(row, col, we, n_dest, n_shard):
    """Sorted-by-dest edges -> 128-dest windows, 128-edge chunks, padded so
    chunk counts per window match across shards (one SPMD program)."""
    window = 128
    order = np.argsort(row, kind="stable")
    row, col, we = row[order], col[order], we[order]
    per = n_dest // n_shard
    nwin = per // window
    counts = np.zeros((n_shard, nwin), np.int64)
    lists = {}
    for s in range(n_shard):
        lo = s * per
        for wi in range(nwin):
            wlo = lo + wi * window
            a = np.searchsorted(row, wlo, side="left")
            b = np.searchsorted(row, wlo + window, side="left")
            lists[(s, wi)] = (row[a:b] - wlo, col[a:b], we[a:b])
            counts[s, wi] = (b - a + 127) // 128
    ncw = np.maximum(counts.max(axis=0), 1)
    C = int(ncw.sum())
    src = np.zeros((n_shard, C, 128), np.int32)
    dst = np.full((n_shard, C, 128), 200.0, np.float32)
    wea = np.zeros((n_shard, C, 128), np.float32)
    for s in range(n_shard):
        base = 0
        for wi in range(nwin):
            dl, cl, wl = lists[(s, wi)]
            n = len(dl)
            k = int(ncw[wi])
            src[s, base:base + k].reshape(-1)[:n] = cl
            dst[s, base:base + k].reshape(-1)[:n] = dl
            wea[s, base:base + k].reshape(-1)[:n] = wl
            base += k
    return [int(x) for x in ncw], src, dst, wea


def _edge_we(e, n):
    row, col = np.asarray(e[0], np.int64), np.asarray(e[1], np.int64)
    deg = np.bincount(row, minlength=n).astype(np.float32)
    dis = np.where(deg > 0, 1.0 / np.sqrt(np.maximum(deg, 1.0)), 0.0).astype(np.float32)
    return row, col, -(dis[row] * dis[col]).astype(np.float32)


def _sub_edges(row, col, we, pool_idx):
    order = np.argsort(row, kind="stable")
    row, col, we = row[order], col[order], we[order]
    starts = np.searchsorted(row, pool_idx, side="left")
    ends = np.searchsorted(row, pool_idx, side="right")
    nr, ncl, nw = [], [], []
    for i in range(len(pool_idx)):
        s, e = starts[i], ends[i]
        if e > s:
            nr.append(np.full(e - s, i, np.int64))
            ncl.append(col[s:e])
            nw.append(we[s:e])
    return np.concatenate(nr), np.concatenate(ncl), np.concatenate(nw)


def _dense_s(row, col, we, n):
    s = np.zeros((n, n), np.float32)
    np.add.at(s, (row, col), we)
    return s


def _tile_w(w, pack):
    """[K, M] -> [K//(128*pack) * 128, pack*M]: pack K-blocks side by side."""
    k, m = w.shape
    nb = k // 128
    t = w.reshape(nb // pack, pack, 128, m).transpose(0, 2, 1, 3)
    return np.ascontiguousarray(t.reshape((nb // pack) * 128, pack * m))


def _host_prep(inputs):
    d = {k: np.asarray(v) for k, v in inputs.items()}
    x = d["x"].astype(np.float32)
    l0 = np.asarray(d["l0"], np.int64)
    l1 = np.asarray(d["l1"], np.int64)
    l2 = np.asarray(d["l2"], np.int64)

    X0 = np.ascontiguousarray(x.transpose(1, 0, 2).reshape(N0, 96))
    X0p = np.zeros((N0, 128), np.float32)
    X0p[:, :96] = X0
    X0l0T = np.ascontiguousarray(X0[l0].T)  # [96, 4096]

    r0, c0, w0 = _edge_we(d["e0"], N0)
    ncw_p1, src_p1, dst_p1, we_p1 = _prep_prop(r0, c0, w0, N0, NCORES)
    r0s, c0s, w0s = _sub_edges(r0, c0, w0, l0)
    ncw_p2, src_p2, dst_p2, we_p2 = _prep_prop(r0s, c0s, w0s, N1, NCORES)

    r1, c1, w1 = _edge_we(d["e1"], N1)
    ncw_q1, src_q1, dst_q1, we_q1 = _prep_prop(r1, c1, w1, N1, 1)
    r1s, c1s, w1s = _sub_edges(r1, c1, w1, l1)
    ncw_q2, src_q2, dst_q2, we_q2 = _prep_prop(r1s, c1s, w1s, N2, 1)

    r2, c2, w2 = _edge_we(d["e2"], N2)
    S2 = _dense_s(r2, c2, w2, N2)
    S2T = _tile_w(np.ascontiguousarray(S2.T), 8)       # [128, 8192]
    S2l2T = _tile_w(np.ascontiguousarray(S2[l2].T), 8)  # [128, 1024]
    P_l2 = np.zeros((N2, 128), np.float32)
    P_l2[l2, np.arange(128)] = 1.0
    P_l2 = _tile_w(P_l2, 8)                             # [128, 1024]

    r3, c3, w3 = _edge_we(d["e3"], N3)
    S3T = np.ascontiguousarray(_dense_s(r3, c3, w3, N3).T)

    def wmod(W):
        return W[0] - W[2], W[1], 2.0 * W[2]

    Wm1 = wmod(d["Wc1"].astype(np.float32))
    Wm = [wmod(d[f"Wc{i}"].astype(np.float32)) for i in (2, 3, 4, 5)]
    eye4 = np.eye(4, dtype=np.float32)

    per_core = []
    for k in range(NCORES):
        m = {}
        m["X0"] = X0p
        m["X0l0T"] = X0l0T
        m["iota"] = np.tile(np.arange(128, dtype=np.float32), (128, 1))
        m["epsv"] = np.full((128, 1), EPS, np.float32)
        m["l0_idx"] = np.ascontiguousarray(
            np.tile(l0.astype(np.int16).reshape(-1, 16).T, (8, 1)))
        m["l1_idx"] = np.ascontiguousarray(
            np.tile(l1.astype(np.int16).reshape(-1, 16).T, (8, 1)))
        for pref, (src, dst, wea) in (
            ("p1", (src_p1[k], dst_p1[k], we_p1[k])),
            ("p2", (src_p2[k], dst_p2[k], we_p2[k])),
            ("q1", (src_q1[0], dst_q1[0], we_q1[0])),
            ("q2", (src_q2[0], dst_q2[0], we_q2[0])),
        ):
            flat = src.reshape(-1).astype(np.int16)
            m[pref + "_srcw"] = np.ascontiguousarray(
                np.tile(flat.reshape(-1, 16).T, (8, 1)))
            m[pref + "_dst"] = np.ascontiguousarray(dst.transpose(1, 0))
            m[pref + "_we"] = np.ascontiguousarray(wea.transpose(1, 0))
        m["S2T"] = S2T
        m["S2l2T"] = S2l2T
        m["P_l2"] = P_l2
        m["S3T"] = S3T
        for t in range(3):
            bw = np.zeros((96, 128), np.float32)
            for j in range(4):
                bg = 4 * k + j
                bw[3 * bg:3 * bg + 3, 32 * j:32 * j + 32] = Wm1[t]
            m[f"bigw0_{t}"] = bw
        for lev in range(4):
            for t in range(3):
                m[f"bigw{lev + 1}_{t}"] = np.kron(eye4, Wm[lev][t])
        for lev, nm in ((1, "b1"), (2, "b2"), (3, "b3"), (4, "b4"), (5, "b5")):
            m[f"bias{lev}"] = np.tile(d[nm].astype(np.float32), 4).reshape(128, 1)
        for li in (6, 7, 8):
            W = d[f"W{li}"].astype(np.float32)[:, 512 * k:512 * k + 512]
            m[f"w{li}"] = _tile_w(W, 8)  # [512, 4096]
            m[f"g{li}"] = np.ascontiguousarray(
                d[f"g{li}"].astype(np.float32)[512 * k:512 * k + 512].reshape(4, 128).T)
            m[f"be{li}"] = np.ascontiguousarray(
                d[f"be{li}"].astype(np.float32)[512 * k:512 * k + 512].reshape(4, 128).T)
        m["w9"] = _tile_w(d["W9"].astype(np.float32)[512 * k:512 * k + 512], 4)  # [128, 512]
        per_core.append(m)

    meta = {"p1": ncw_p1, "p2": ncw_p2, "q1": ncw_q1, "q2": ncw_q2}
    return per_core, meta


# ---------------------------------------------------------------- device program
def _build_nc(meta, shapes):
    nc = bacc.Bacc("TRN2", target_bir_lowering=False, debug=False, num_devices=NCORES)
    ein = {}
    for name, arr in shapes.items():
        dt = {np.dtype(np.int32): I32, np.dtype(np.int16): I16}.get(arr.dtype, F32)
        ein[name] = nc.dram_tensor(name, list(arr.shape), dt, kind="ExternalInput")
    out_mu = nc.dram_tensor("mu", [128, 32], F32, kind="ExternalOutput")

    tx1_loc = nc.dram_tensor("tx1_loc", [N0 // 8, 128], F32)
    tx1_all = nc.dram_tensor("tx1_all", [N0, 128], F32)
    p2t_loc = nc.dram_tensor("p2t_loc", [96, 512], F32)
    p2t_all = nc.dram_tensor("p2t_all", [8 * 96, 512], F32)
    z1_dram = nc.dram_tensor("z1_dram", [N1, 128], F32)
    t1l1_dram = nc.dram_tensor("t1l1_dram", [N1, 128], F32)
    x6_loc = nc.dram_tensor("x6_loc", [4096, 4], F32)
    x6_all = nc.dram_tensor("x6_all", [8 * 4096, 4], F32)
    h6_loc = nc.dram_tensor("h6_loc", [512, 32], F32)
    h6_all = nc.dram_tensor("h6_all", [4096, 32], F32)
    h7_loc = nc.dram_tensor("h7_loc", [512, 32], F32)
    h7_all = nc.dram_tensor("h7_all", [4096, 32], F32)
    mu_loc = nc.dram_tensor("mu_loc", [128, 32], F32)
    mu_all = nc.dram_tensor("mu_all", [8 * 128, 32], F32)

    with tile.TileContext(nc) as tc:
        with (
            tc.tile_pool(name="const", bufs=1) as cpool,
            tc.tile_pool(name="big", bufs=1) as bigpool,
            tc.tile_pool(name="work", bufs=3) as wpool,
            tc.tile_pool(name="wload", bufs=2) as wlpool,
            tc.tile_pool(name="psA", bufs=3, space="PSUM") as ppool,
            tc.tile_pool(name="psB", bufs=1, space="PSUM") as apool,
        ):
            ident = cpool.tile([128, 128], F32, tag="ident", name="ident")
            make_identity(nc, ident[:])
            iota_t = cpool.tile([128, 128], F32, tag="iota", name="iota")
            nc.sync.dma_start(out=iota_t[:], in_=ein["iota"][:, :])
            eps_t = cpool.tile([128, 1], F32, tag="epsv", name="epsv")
            nc.sync.dma_start(out=eps_t[:], in_=ein["epsv"][:, :])

            def load_const(name):
                t = cpool.tile(list(shapes[name].shape), F32, tag=name)
                nc.sync.dma_start(out=t[:], in_=ein[name][:, :])
                return t

            def load_chunk_arrs(pref, C):
                s = cpool.tile([128, C * 8], I16, tag=pref + "s", name=pref + "s")
                dd = cpool.tile([128, C], F32, tag=pref + "d", name=pref + "d")
                w = cpool.tile([128, C], F32, tag=pref + "w", name=pref + "w")
                nc.sync.dma_start(out=s[:], in_=ein[pref + "_srcw"][:, :])
                nc.sync.dma_start(out=dd[:], in_=ein[pref + "_dst"][:, :])
                nc.sync.dma_start(out=w[:], in_=ein[pref + "_we"][:, :])
                return s, dd, w

            GRP = 16

            def grp_gather(idx_sb, g0, gc, gather_src):
                zb = wpool.tile([128, GRP * 128], F32, tag="zb", name="zb", bufs=3)
                nc.gpsimd.dma_gather(
                    out_ap=zb[:, :gc * 128].rearrange("p (c e) -> p c e", e=128),
                    in_ap=gather_src[:, :],
                    idxs_ap=idx_sb[:, g0 * 8:(g0 + gc) * 8],
                    num_idxs=gc * 128, num_idxs_reg=gc * 128, elem_size=128,
                    single_packet=False)
                return zb

            def mk_sel(eng, dst_ap, we_ap):
                sel = wpool.tile([128, 128], F32, tag="sel", name="sel")
                eng.tensor_scalar(out=sel[:], in0=iota_t[:], scalar1=dst_ap,
                                  scalar2=we_ap, op0=ALU.is_equal, op1=ALU.mult)
                return sel

            def prop_nodemajor(ncw, pref, gather_src, D, evac):
                C = sum(ncw)
                s, dd, w = load_chunk_arrs(pref, C)
                zbs = {}
                for g0 in range(0, C, GRP):
                    gc = min(GRP, C - g0)
                    zbs[g0] = grp_gather(s, g0, gc, gather_src)
                base = 0
                for wi, nch in enumerate(ncw):
                    ps = ppool.tile([128, 512], F32, tag="ps", name="ps")
                    for c in range(nch):
                        cc = base + c
                        zb = zbs[(cc // GRP) * GRP]
                        lo = (cc % GRP) * 128
                        sel = mk_sel(nc.vector, dd[:, cc:cc + 1], w[:, cc:cc + 1])
                        nc.tensor.matmul(out=ps[:, :D], lhsT=sel[:],
                                         rhs=zb[:, lo:lo + D],
                                         start=(c == 0), stop=(c == nch - 1))
                    evac(wi, ps[:, :D])
                    base += nch

            def transp(src_ap, dst_ap):
                p, f = src_ap.shape
                ps = ppool.tile([128, 512], F32, tag="ps", name="ps")
                nc.tensor.transpose(out=ps[:f, :p], in_=src_ap, identity=ident[:])
                nc.scalar.activation(out=dst_ap, in_=ps[:f, :p], func=AF.Copy)

            def gather_T(idx_t, chunks, gather_src, D, outT):
                chunks = list(chunks)
                zb = grp_gather(idx_t, chunks[0], len(chunks), gather_src)
                for ci in range(len(chunks)):
                    transp(zb[:, ci * 128:ci * 128 + D],
                           outT[:, ci * 128:(ci + 1) * 128])

            def einsum_win(bigw, taps, Din, width, out_ap, func, bias_ap):
                ps = ppool.tile([128, 512], F32, tag="ps", name="ps")
                for t in range(3):
                    nc.tensor.matmul(out=ps[:, :width], lhsT=bigw[t][:Din, :],
                                     rhs=taps[t], start=(t == 0), stop=(t == 2))
                f2 = AF.Identity if func == AF.Copy else func
                nc.scalar.activation(out=out_ap, in_=ps[:, :width], func=f2, bias=bias_ap)

            # ================= LEVEL 0 =================
            with nc.named_scope("l0_prop1"):
                def evac_p1(wi, ps_ap):
                    t = wpool.tile([128, 96], F32, tag="ev", name="ev", bufs=6)
                    nc.scalar.activation(out=t[:], in_=ps_ap, func=AF.Copy)
                    nc.sync.dma_start(out=tx1_loc[wi * 128:(wi + 1) * 128, :96], in_=t[:])
                prop_nodemajor(meta["p1"], "p1", ein["X0"], 96, evac_p1)
            with nc.named_scope("ag1"):
                nc.gpsimd.collective_compute(
                    "AllGather", ALU.bypass, replica_groups=RG,
                    ins=[tx1_loc.ap().opt()], outs=[tx1_all.ap().opt()])

            with nc.named_scope("l0_prop2"):
                C2 = sum(meta["p2"])
                s2c, d2c, w2c = load_chunk_arrs("p2", C2)
                zbs2 = {}
                for g0 in range(0, C2, GRP):
                    gc = min(GRP, C2 - g0)
                    zbs2[g0] = grp_gather(s2c, g0, gc, tx1_all)
                p2t_sb = bigpool.tile([96, 512], F32, tag="p2t_sb", name="p2t_sb")
                base = 0
                for wi, nch in enumerate(meta["p2"]):
                    ps = ppool.tile([128, 512], F32, tag="ps", name="ps")
                    for c in range(nch):
                        cc = base + c
                        zb = zbs2[(cc // GRP) * GRP]
                        lo = (cc % GRP) * 128
                        sel = mk_sel(nc.vector, d2c[:, cc:cc + 1], w2c[:, cc:cc + 1])
                        nc.tensor.matmul(out=ps[:96, :128],
                                         lhsT=zb[:, lo:lo + 96], rhs=sel[:],
                                         start=(c == 0), stop=(c == nch - 1))
                    nc.scalar.activation(out=p2t_sb[:, wi * 128:(wi + 1) * 128],
                                         in_=ps[:96, :128], func=AF.Copy)
                    base += nch
                nc.sync.dma_start(out=p2t_loc[:, :], in_=p2t_sb[:])
            with nc.named_scope("ag2"):
                nc.gpsimd.collective_compute(
                    "AllGather", ALU.bypass, replica_groups=RG,
                    ins=[p2t_loc.ap().opt()], outs=[p2t_all.ap().opt()])

            with nc.named_scope("l0_einsum"):
                l0i = cpool.tile([128, 32 * 8], I16, tag="l0i", name="l0i")
                nc.sync.dma_start(out=l0i[:], in_=ein["l0_idx"][:, :])
                bw0 = [load_const(f"bigw0_{t}") for t in range(3)]
                bias1 = load_const("bias1")
                for w in range(8):
                    g0w = wpool.tile([96, 512], F32, tag="g0w", name="g0w")
                    nc.sync.dma_start(out=g0w[:], in_=ein["X0l0T"][:, 512 * w:512 * (w + 1)])
                    g1w = wpool.tile([96, 512], F32, tag="g1w", name="g1w")
                    gather_T(l0i, range(4 * w, 4 * w + 4), tx1_all, 96, g1w)
                    p2w = wpool.tile([96, 512], F32, tag="p2w", name="p2w")
                    nc.sync.dma_start(out=p2w[:], in_=p2t_all[96 * w:96 * (w + 1), :])
                    z1Tw = wpool.tile([128, 512], F32, tag="z1Tw", name="z1Tw")
                    einsum_win(bw0, [g0w[:], g1w[:], p2w[:]], 96, 512,
                               z1Tw[:], AF.Copy, bias1[:, 0:1])
                    for c in range(4):
                        t = wpool.tile([128, 128], F32, tag="z1nc", name="z1nc")
                        transp(z1Tw[:, c * 128:(c + 1) * 128], t[:])
                        r = w * 512 + c * 128
                        nc.sync.dma_start(out=z1_dram[r:r + 128, :], in_=t[:])

            # ================= LEVEL 1 =================
            with nc.named_scope("l1_prop1"):
                def evac_q1(wi, ps_ap):
                    t = wpool.tile([128, 128], F32, tag="ev", name="ev", bufs=6)
                    nc.scalar.activation(out=t[:], in_=ps_ap, func=AF.Copy)
                    nc.sync.dma_start(out=t1l1_dram[wi * 128:(wi + 1) * 128, :], in_=t[:])
                prop_nodemajor(meta["q1"], "q1", z1_dram, 128, evac_q1)

            p2n_l1 = bigpool.tile([128, 8 * 128], F32, tag="p2n_l1", name="p2n_l1")
            with nc.named_scope("l1_prop2"):
                def evac_q2(wi, ps_ap):
                    nc.scalar.activation(out=p2n_l1[:, wi * 128:(wi + 1) * 128],
                                         in_=ps_ap, func=AF.Copy)
                prop_nodemajor(meta["q2"], "q2", t1l1_dram, 128, evac_q2)

            z2n = bigpool.tile([128, 8 * 128], F32, tag="z2n", name="z2n")
            with nc.named_scope("l1_einsum"):
                l1i = cpool.tile([128, 8 * 8], I16, tag="l1i", name="l1i")
                nc.sync.dma_start(out=l1i[:], in_=ein["l1_idx"][:, :])
                z1l1T = bigpool.tile([128, 1024], F32, tag="z1l1T", name="z1l1T")
                gather_T(l1i, range(8), z1_dram, 128, z1l1T)
                t1l1T = bigpool.tile([128, 1024], F32, tag="t1l1T", name="t1l1T")
                gather_T(l1i, range(8), t1l1_dram, 128, t1l1T)
                p2l1T = bigpool.tile([128, 1024], F32, tag="p2l1T", name="p2l1T")
                for c in range(8):
                    transp(p2n_l1[:, c * 128:(c + 1) * 128], p2l1T[:, c * 128:(c + 1) * 128])
                bw1 = [load_const(f"bigw1_{t}") for t in range(3)]
                bias2 = load_const("bias2")
                z2T = bigpool.tile([128, 1024], F32, tag="z2T", name="z2T")
                for w in range(2):
                    einsum_win(bw1, [z1l1T[:, 512 * w:512 * (w + 1)],
                                     t1l1T[:, 512 * w:512 * (w + 1)],
                                     p2l1T[:, 512 * w:512 * (w + 1)]],
                               128, 512, z2T[:, 512 * w:512 * (w + 1)], AF.Tanh, bias2[:, 0:1])
                for c in range(8):
                    transp(z2T[:, c * 128:(c + 1) * 128], z2n[:, c * 128:(c + 1) * 128])

            # ================= LEVEL 2 (dense) =================
            with nc.named_scope("l2"):
                t1_l2 = bigpool.tile([128, 8 * 128], F32, tag="t1_l2", name="t1_l2")
                for half in range(2):
                    s2t = wlpool.tile([128, 4096], F32, tag="wld", name="wld")
                    nc.sync.dma_start(out=s2t[:], in_=ein["S2T"][:, 4096 * half:4096 * (half + 1)])
                    for dc in range(8):
                        ps = ppool.tile([128, 512], F32, tag="ps", name="ps")
                        for kk in range(4):
                            kc = half * 4 + kk
                            nc.tensor.matmul(
                                out=ps[:, :128],
                                lhsT=s2t[:, kk * 1024 + dc * 128: kk * 1024 + dc * 128 + 128],
                                rhs=z2n[:, kc * 128:(kc + 1) * 128],
                                start=(kk == 0), stop=(kk == 3))
                        if half == 0:
                            nc.scalar.activation(out=t1_l2[:, dc * 128:(dc + 1) * 128],
                                                 in_=ps[:, :128], func=AF.Copy)
                        else:
                            nc.vector.tensor_add(t1_l2[:, dc * 128:(dc + 1) * 128],
                                                 t1_l2[:, dc * 128:(dc + 1) * 128],
                                                 ps[:, :128])
                s2l2 = cpool.tile([128, 1024], F32, tag="s2l2", name="s2l2")
                nc.sync.dma_start(out=s2l2[:], in_=ein["S2l2T"][:, :])
                ps = ppool.tile([128, 512], F32, tag="ps", name="ps")
                for kc in range(8):
                    nc.tensor.matmul(out=ps[:, :128], lhsT=s2l2[:, kc * 128:(kc + 1) * 128],
                                     rhs=t1_l2[:, kc * 128:(kc + 1) * 128],
                                     start=(kc == 0), stop=(kc == 7))
                p2n_l2 = wpool.tile([128, 128], F32, tag="p2n_l2", name="p2n_l2")
                nc.scalar.activation(out=p2n_l2[:], in_=ps[:, :128], func=AF.Copy)
                pl2 = cpool.tile([128, 1024], F32, tag="pl2", name="pl2")
                nc.sync.dma_start(out=pl2[:], in_=ein["P_l2"][:, :])
                z2l2T = wpool.tile([128, 128], F32, tag="z2l2T", name="z2l2T")
                psg = ppool.tile([128, 512], F32, tag="ps", name="ps")
                for kc in range(8):
                    nc.tensor.matmul(out=psg[:, :128], lhsT=z2n[:, kc * 128:(kc + 1) * 128],
                                     rhs=pl2[:, kc * 128:(kc + 1) * 128],
                                     start=(kc == 0), stop=(kc == 7))
                nc.scalar.activation(out=z2l2T[:], in_=psg[:, :128], func=AF.Copy)
                t1l2T = wpool.tile([128, 128], F32, tag="t1l2T", name="t1l2T")
                psg2 = ppool.tile([128, 512], F32, tag="ps", name="ps")
                for kc in range(8):
                    nc.tensor.matmul(out=psg2[:, :128], lhsT=t1_l2[:, kc * 128:(kc + 1) * 128],
                                     rhs=pl2[:, kc * 128:(kc + 1) * 128],
                                     start=(kc == 0), stop=(kc == 7))
                nc.scalar.activation(out=t1l2T[:], in_=psg2[:, :128], func=AF.Copy)
                p2l2T = wpool.tile([128, 128], F32, tag="p2l2T", name="p2l2T")
                transp(p2n_l2[:], p2l2T[:])
                bw2 = [load_const(f"bigw2_{t}") for t in range(3)]
                bias3 = load_const("bias3")
                z3T = wpool.tile([128, 128], F32, tag="z3T", name="z3T")
                einsum_win(bw2, [z2l2T[:], t1l2T[:], p2l2T[:]], 128, 128,
                           z3T[:], AF.Tanh, bias3[:, 0:1])
                z3n = wpool.tile([128, 128], F32, tag="z3n", name="z3n")
                transp(z3T[:], z3n[:])

            # ================= LEVEL 3 =================
            with nc.named_scope("l3"):
                s3t = cpool.tile([128, 128], F32, tag="s3t", name="s3t")
                nc.sync.dma_start(out=s3t[:], in_=ein["S3T"][:, :])
                bias4 = load_const("bias4")
                bias5 = load_const("bias5")

                def conv_l3(zn, zT, bw_pref, bias_t, func, keep):
                    t1T = wpool.tile([128, 128], F32, tag=keep + "t1T", name=keep + "t1T")
                    ps = ppool.tile([128, 512], F32, tag="ps", name="ps")
                    nc.tensor.matmul(out=ps[:, :128], lhsT=zn, rhs=s3t[:], start=True, stop=True)
                    nc.scalar.activation(out=t1T[:], in_=ps[:, :128], func=AF.Copy)
                    t1n_ = wpool.tile([128, 128], F32, tag=keep + "t1n", name=keep + "t1n")
                    transp(t1T[:], t1n_[:])
                    p2T_ = wpool.tile([128, 128], F32, tag=keep + "p2T", name=keep + "p2T")
                    ps2 = ppool.tile([128, 512], F32, tag="ps", name="ps")
                    nc.tensor.matmul(out=ps2[:, :128], lhsT=t1n_[:], rhs=s3t[:], start=True, stop=True)
                    nc.scalar.activation(out=p2T_[:], in_=ps2[:, :128], func=AF.Copy)
                    bw = [load_const(f"{bw_pref}_{t}") for t in range(3)]
                    outT = wpool.tile([128, 128], F32, tag=keep + "oT", name=keep + "oT")
                    einsum_win(bw, [zT, t1T[:], p2T_[:]], 128, 128, outT[:], func, bias_t[:, 0:1])
                    outn = wpool.tile([128, 128], F32, tag=keep + "on", name=keep + "on")
                    transp(outT[:], outn[:])
                    return outn, outT

                z4n, z4T = conv_l3(z3n[:], z3T[:], "bigw3", bias4, AF.Tanh, "c4")
                o5n, o5T = conv_l3(z4n[:], z4T[:], "bigw4", bias5, AF.Copy, "c5")

            # ================= MLP input assembly =================
            with nc.named_scope("mlp_in"):
                for j in range(4):
                    ap_out = x6_loc.ap()[:, j:j + 1].rearrange("(n h) o -> n (h o)", h=32)
                    nc.sync.dma_start(out=ap_out, in_=o5n[:, 32 * j:32 * j + 32])
                nc.gpsimd.collective_compute(
                    "AllGather", ALU.bypass, replica_groups=RG,
                    ins=[x6_loc.ap().opt()], outs=[x6_all.ap().opt()])

            # ================= MLP =================
            def mlp_layer(nm, src_sb, out_sb):
                g_t = load_const("g" + nm[1])
                be_t = load_const("be" + nm[1])
                pss = [apool.tile([128, 32], F32, tag=f"acc{m}", name=f"acc{m}") for m in range(4)]
                for i in range(4):
                    wt = wlpool.tile([128, 4096], F32, tag="wld", name="wld")
                    nc.sync.dma_start(out=wt[:], in_=ein[nm][128 * i:128 * (i + 1), :])
                    for a in range(8):
                        kc = i * 8 + a
                        for mm in range(4):
                            nc.tensor.matmul(
                                out=pss[mm][:],
                                lhsT=wt[:, a * 512 + mm * 128: a * 512 + mm * 128 + 128],
                                rhs=src_sb[:, 32 * kc:32 * kc + 32],
                                start=(kc == 0), stop=(kc == 31))
                for mm in range(4):
                    t = wpool.tile([128, 32], F32, tag="b_t", name="b_t")
                    nc.vector.tensor_copy(t[:], pss[mm][:])
                    s1 = wpool.tile([128, 1], F32, tag="b_s1", name="b_s1")
                    nc.vector.tensor_reduce(out=s1[:], in_=t[:], axis=AX.X, op=ALU.add)
                    mu_ = wpool.tile([128, 1], F32, tag="b_mu", name="b_mu")
                    nc.vector.tensor_scalar_mul(mu_[:], s1[:], 1.0 / 32.0)
                    sq = wpool.tile([128, 32], F32, tag="b_sq", name="b_sq")
                    nc.vector.tensor_mul(sq[:], t[:], t[:])
                    s2_ = wpool.tile([128, 1], F32, tag="b_s2", name="b_s2")
                    nc.vector.tensor_reduce(out=s2_[:], in_=sq[:], axis=AX.X, op=ALU.add)
                    var = wpool.tile([128, 1], F32, tag="b_var", name="b_var")
                    nc.vector.scalar_tensor_tensor(out=var[:], in0=mu_[:], scalar=-1.0,
                                                   in1=mu_[:], op0=ALU.mult, op1=ALU.mult)
                    nc.vector.scalar_tensor_tensor(out=var[:], in0=s2_[:], scalar=1.0 / 32.0,
                                                   in1=var[:], op0=ALU.mult, op1=ALU.add)
                    sd = wpool.tile([128, 1], F32, tag="b_sd", name="b_sd")
                    nc.scalar.activation(out=sd[:], in_=var[:], func=AF.Sqrt, bias=eps_t[:, 0:1])
                    rs = wpool.tile([128, 1], F32, tag="b_rs", name="b_rs")
                    nc.vector.reciprocal(rs[:], sd[:])
                    a_ = wpool.tile([128, 1], F32, tag="b_a", name="b_a")
                    nc.vector.tensor_mul(a_[:], rs[:], g_t[:, mm:mm + 1])
                    sh = wpool.tile([128, 1], F32, tag="b_sh", name="b_sh")
                    nc.vector.scalar_tensor_tensor(out=sh[:], in0=mu_[:], scalar=-1.0,
                                                   in1=a_[:], op0=ALU.mult, op1=ALU.mult)
                    nc.vector.tensor_add(sh[:], sh[:], be_t[:, mm:mm + 1])
                    nc.scalar.activation(out=out_sb[:, 32 * mm:32 * mm + 32], in_=t[:],
                                         func=AF.Relu, scale=a_[:, 0:1], bias=sh[:, 0:1])

            with nc.named_scope("mlp6"):
                x6T = bigpool.tile([128, 1024], F32, tag="x6T", name="x6T")
                for kk in range(8):
                    nc.sync.dma_start(
                        out=x6T[:].rearrange("p (c r) -> p c r", r=32)[:, :, 4 * kk:4 * kk + 4],
                        in_=x6_all[4096 * kk:4096 * (kk + 1), :].rearrange(
                            "(c p) j -> p c j", p=128))
                h6 = bigpool.tile([128, 128], F32, tag="h6sb", name="h6sb")
                mlp_layer("w6", x6T, h6)
                nc.sync.dma_start(out=h6_loc.ap().rearrange("(m p) b -> p m b", p=128),
                                  in_=h6[:].rearrange("p (m b) -> p m b", b=32))
                nc.gpsimd.collective_compute(
                    "AllGather", ALU.bypass, replica_groups=RG,
                    ins=[h6_loc.ap().opt()], outs=[h6_all.ap().opt()])
            with nc.named_scope("mlp7"):
                x7T = bigpool.tile([128, 1024], F32, tag="x7T", name="x7T")
                nc.sync.dma_start(out=x7T[:].rearrange("p (c b) -> p c b", b=32),
                                  in_=h6_all[:, :].rearrange("(c p) b -> p c b", p=128))
                h7 = bigpool.tile([128, 128], F32, tag="h7sb", name="h7sb")
                mlp_layer("w7", x7T, h7)
                nc.sync.dma_start(out=h7_loc.ap().rearrange("(m p) b -> p m b", p=128),
                                  in_=h7[:].rearrange("p (m b) -> p m b", b=32))
                nc.gpsimd.collective_compute(
                    "AllGather", ALU.bypass, replica_groups=RG,
                    ins=[h7_loc.ap().opt()], outs=[h7_all.ap().opt()])
            with nc.named_scope("mlp8"):
                x8T = bigpool.tile([128, 1024], F32, tag="x8T", name="x8T")
                nc.sync.dma_start(out=x8T[:].rearrange("p (c b) -> p c b", b=32),
                                  in_=h7_all[:, :].rearrange("(c p) b -> p c b", p=128))
                h8 = bigpool.tile([128, 128], F32, tag="h8sb", name="h8sb")
                mlp_layer("w8", x8T, h8)

            with nc.named_scope("mlp9"):
                w9t = cpool.tile([128, 512], F32, tag="w9t", name="w9t")
                nc.sync.dma_start(out=w9t[:], in_=ein["w9"][:, :])
                ps9 = apool.tile([128, 32], F32, tag="acc0", name="acc0")
                for kc in range(4):
                    nc.tensor.matmul(out=ps9[:], lhsT=w9t[:, kc * 128:(kc + 1) * 128],
                                     rhs=h8[:, 32 * kc:32 * kc + 32],
                                     start=(kc == 0), stop=(kc == 3))
                mu_sb = wpool.tile([128, 32], F32, tag="mu_sb", name="mu_sb")
                nc.scalar.activation(out=mu_sb[:], in_=ps9[:], func=AF.Copy)
                nc.sync.dma_start(out=mu_loc[:, :], in_=mu_sb[:])
                nc.gpsimd.collective_compute(
                    "AllGather", ALU.bypass, replica_groups=RG,
                    ins=[mu_loc.ap().opt()], outs=[mu_all.ap().opt()])
                tot = wpool.tile([128, 32], F32, tag="f_tot", name="f_tot")
                nc.sync.dma_start(out=tot[:], in_=mu_all[0:128, :])
                for k in range(1, 8):
                    pk = wpool.tile([128, 32], F32, tag="f_pk", name="f_pk")
                    nc.sync.dma_start(out=pk[:], in_=mu_all[k * 128:(k + 1) * 128, :])
                    nc.vector.tensor_add(tot[:], tot[:], pk[:])
                s1 = wpool.tile([128, 1], F32, tag="f_s1", name="f_s1")
                nc.vector.tensor_reduce(out=s1[:], in_=tot[:], axis=AX.X, op=ALU.add)
                mu_ = wpool.tile([128, 1], F32, tag="f_mu", name="f_mu")
                nc.vector.tensor_scalar_mul(mu_[:], s1[:], 1.0 / 32.0)
                sq = wpool.tile([128, 32], F32, tag="f_sq", name="f_sq")
                nc.vector.tensor_mul(sq[:], tot[:], tot[:])
                s2_ = wpool.tile([128, 1], F32, tag="f_s2", name="f_s2")
                nc.vector.tensor_reduce(out=s2_[:], in_=sq[:], axis=AX.X, op=ALU.add)
                var = wpool.tile([128, 1], F32, tag="f_var", name="f_var")
                nc.vector.scalar_tensor_tensor(out=var[:], in0=mu_[:], scalar=-1.0,
                                               in1=mu_[:], op0=ALU.mult, op1=ALU.mult)
                nc.vector.scalar_tensor_tensor(out=var[:], in0=s2_[:], scalar=1.0 / 32.0,
                                               in1=var[:], op0=ALU.mult, op1=ALU.add)
                sdf = wpool.tile([128, 1], F32, tag="f_sd", name="f_sd")
                nc.scalar.activation(out=sdf[:], in_=var[:], func=AF.Sqrt, bias=eps_t[:, 0:1])
                rs = wpool.tile([128, 1], F32, tag="f_rs", name="f_rs")
                nc.vector.reciprocal(rs[:], sdf[:])
                neg = wpool.tile([128, 1], F32, tag="f_neg", name="f_neg")
                nc.vector.scalar_tensor_tensor(out=neg[:], in0=mu_[:], scalar=-1.0,
                                               in1=rs[:], op0=ALU.mult, op1=ALU.mult)
                outt = wpool.tile([128, 32], F32, tag="f_out", name="f_out")
                nc.scalar.activation(out=outt[:], in_=tot[:], func=AF.Identity,
                                     scale=rs[:, 0:1], bias=neg[:, 0:1])
                nc.sync.dma_start(out=out_mu[:, :], in_=outt[:])

    nc.compile()
    return nc


# ---------------------------------------------------------------- entry point
def kernel(**inputs) -> np.ndarray:
    per_core, meta = _host_prep(inputs)
    if "prog" not in _CACHE:
        _CACHE["prog"] = _build_nc(meta, per_core[0])
    nc = _CACHE["prog"]
    res = bass_utils.run_bass_kernel_spmd(nc, per_core, core_ids=list(range(NCORES)))
    return np.ascontiguousarray(res.results[0]["mu"].T)



# revision 9
# speedup vs baseline: 2.4811x; 2.4811x over previous
"""Trainium2 Bass kernel for nn_Encoder_base (5x ChebConv GNN + pool + MLP).

Restructured for speed:
  - Only pooled rows of each ChebConv are needed downstream, so each level's
    Chebyshev taps are computed directly at the pooled nodes via
    host-precomputed operators: tap1 = S[pool,:] z, tap2 = (S^2)[pool,:] z.
  - Level 0 applies them as per-edge selection matmuls whose source rows are
    HOST-gathered from x (x is a kernel input), streamed with the weighted
    selection matrices as one bf16 stream -> zero device gathers, zero DVE.
  - Levels 1-2 apply the pooled operators as dense bf16 matmuls; level 3
    (no pooling) uses dense S3 / S3^2.
  - Everything on the matmul path is bf16 (fp32 matmul is 4 cycles/row on
    TRN2, bf16 is 1 and gets fast weight load).
  - Distribution: level 0 edge-sharded by pooled dest (one small AllGather
    of the taps); levels 1-3 batch-sharded (4 batches/core, replicated
    small dense ops); MLP output-feature sharded with per-layer AllGathers.
"""
import numpy as np
import ml_dtypes
import concourse.bass as bass
import concourse.bacc as bacc
import concourse.tile as tile
from concourse import mybir, bass_utils

F32 = mybir.dt.float32
BF16 = mybir.dt.bfloat16
I16 = mybir.dt.int16
NPBF = ml_dtypes.bfloat16
AF = mybir.ActivationFunctionType
ALU = mybir.AluOpType
AX = mybir.AxisListType
RG = [list(range(8))]
NCORES = 8
N0, N1, N2, N3 = 16384, 4096, 1024, 128
EPS = 1e-5

_CACHE = {}


# ---------------------------------------------------------------- host prep
def _edge_we(e, n):
    row = np.asarray(e[0], np.int64)
    col = np.asarray(e[1], np.int64)
    deg = np.bincount(row, minlength=n).astype(np.float32)
    dis = np.where(deg > 0, 1.0 / np.sqrt(np.maximum(deg, 1.0)), 0.0).astype(np.float32)
    return row, col, -(dis[row] * dis[col]).astype(np.float32)


def _sort_by_row(row, col, we):
    order = np.argsort(row, kind="stable")
    return row[order], col[order], we[order]


def _sub_edges(row, col, we, pool_idx):
    row, col, we = _sort_by_row(row, col, we)
    starts = np.searchsorted(row, pool_idx, side="left")
    ends = np.searchsorted(row, pool_idx, side="right")
    cnt = ends - starts
    tot = int(cnt.sum())
    pos = np.repeat(starts, cnt) + (np.arange(tot) - np.repeat(np.cumsum(cnt) - cnt, cnt))
    nr = np.repeat(np.arange(len(pool_idx), dtype=np.int64), cnt)
    return nr, col[pos], we[pos]


def _two_hop(rd, cd, wd, row, col, we, n):
    row, col, we = _sort_by_row(row, col, we)
    indptr = np.searchsorted(row, np.arange(n + 1))
    s, e = indptr[cd], indptr[cd + 1]
    cnt = e - s
    tot = int(cnt.sum())
    pos = np.repeat(s, cnt) + (np.arange(tot) - np.repeat(np.cumsum(cnt) - cnt, cnt))
    return np.repeat(rd, cnt), col[pos], np.repeat(wd, cnt) * we[pos]


def _dense_op(rd, cd, wd, n_rows, n_cols):
    m = np.zeros((n_rows, n_cols), np.float32)
    np.add.at(m, (rd, cd), wd)
    return m


def _prep_l0_streams(ops, x_rows, n_dest, window=128):
    """Per-edge (xg | sel) bf16 streams, dest-sharded over cores."""
    per = n_dest // NCORES
    nwin = per // window
    feat = x_rows.shape[1]
    ncw_per_op, lists = [], {}
    for oi, (rd, cd, wd) in enumerate(ops):
        rd, cd, wd = _sort_by_row(rd, cd, wd)
        counts = np.zeros((NCORES, nwin), np.int64)
        for k in range(NCORES):
            lo = k * per
            for wi in range(nwin):
                a = np.searchsorted(rd, lo + wi * window, side="left")
                b = np.searchsorted(rd, lo + (wi + 1) * window, side="left")
                lists[(oi, k, wi)] = (rd[a:b] - (lo + wi * window), cd[a:b], wd[a:b])
                counts[k, wi] = (b - a + 127) // 128
        ncw_per_op.append(np.maximum(counts.max(axis=0), 1).astype(np.int64))
    ctot = int(sum(n.sum() for n in ncw_per_op))
    win_of_chunk = []
    for oi in range(len(ops)):
        for wi in range(nwin):
            win_of_chunk += [wi] * int(ncw_per_op[oi][wi])
    xgsel = []
    for k in range(NCORES):
        xg = np.zeros((ctot * 128, 128), NPBF)
        sel = np.zeros((ctot * 128, window), NPBF)
        cbase = 0
        for oi in range(len(ops)):
            for wi in range(nwin):
                dl, cl, wl = lists[(oi, k, wi)]
                ne = len(dl)
                e0 = cbase * 128
                xg[e0:e0 + ne, :feat] = x_rows[cl]
                sel[e0 + np.arange(ne), dl] = wl.astype(NPBF)
                cbase += int(ncw_per_op[oi][wi])
        xg = xg.reshape(ctot, 128, 128).transpose(1, 0, 2)
        sel = sel.reshape(ctot, 128, window).transpose(1, 0, 2)
        comb = np.concatenate([xg, sel], axis=2)
        xgsel.append(np.ascontiguousarray(comb.reshape(128, ctot * 256)))
    return [list(map(int, n)) for n in ncw_per_op], win_of_chunk, xgsel


def _tile_rows(mat, tl=128):
    n, c = mat.shape
    nt = n // tl
    return np.ascontiguousarray(
        mat.reshape(nt, tl, c).transpose(1, 0, 2).reshape(tl, nt * c))


def _tile_w(w, pack):
    k, m = w.shape
    nb = k // 128
    t = w.reshape(nb // pack, pack, 128, m).transpose(0, 2, 1, 3)
    return np.ascontiguousarray(t.reshape((nb // pack) * 128, pack * m))


def _idx16(idx):
    return np.ascontiguousarray(
        np.tile(idx.astype(np.int16).reshape(-1, 16).T, (8, 1)))


def _wmod(W):
    W = W.astype(np.float32)
    return W[0] - W[2], W[1], 2.0 * W[2]


def _host_prep(inputs):
    d = {k: np.asarray(v) for k, v in inputs.items()}
    x = d["x"].astype(np.float32)
    l0 = np.asarray(d["l0"], np.int64)
    l1 = np.asarray(d["l1"], np.int64)
    l2 = np.asarray(d["l2"], np.int64)

    X0 = np.ascontiguousarray(x.transpose(1, 0, 2).reshape(N0, 96))
    X0b = X0.astype(NPBF)

    r0, c0, w0 = _edge_we(d["e0"], N0)
    rd0, cd0, wd0 = _sub_edges(r0, c0, w0, l0)
    rm0, cm0, wm0 = _two_hop(rd0, cd0, wd0, r0, c0, w0, N0)
    ncw_ops, win_of_chunk, xgsel = _prep_l0_streams(
        [(rd0, cd0, wd0), (rm0, cm0, wm0)], X0b, N1)

    r1, c1, w1 = _edge_we(d["e1"], N1)
    rd1, cd1, wd1 = _sub_edges(r1, c1, w1, l1)
    rm1, cm1, wm1 = _two_hop(rd1, cd1, wd1, r1, c1, w1, N1)
    d1t = _tile_rows(np.ascontiguousarray(
        _dense_op(rd1, cd1, wd1, N2, N1).T).astype(NPBF))
    m1t = _tile_rows(np.ascontiguousarray(
        _dense_op(rm1, cm1, wm1, N2, N1).T).astype(NPBF))

    r2, c2, w2 = _edge_we(d["e2"], N2)
    rd2, cd2, wd2 = _sub_edges(r2, c2, w2, l2)
    rm2, cm2, wm2 = _two_hop(rd2, cd2, wd2, r2, c2, w2, N2)
    d2t = _tile_rows(np.ascontiguousarray(
        _dense_op(rd2, cd2, wd2, N3, N2).T).astype(NPBF))
    m2t = _tile_rows(np.ascontiguousarray(
        _dense_op(rm2, cm2, wm2, N3, N2).T).astype(NPBF))

    r3, c3, w3 = _edge_we(d["e3"], N3)
    S3 = _dense_op(r3, c3, w3, N3, N3)
    s3t = np.ascontiguousarray(S3.T).astype(NPBF)
    s3sqt = np.ascontiguousarray((S3 @ S3).T).astype(NPBF)

    Wm0 = _wmod(d["Wc1"])
    Wms = [_wmod(d[f"Wc{i}"]) for i in (2, 3, 4, 5)]
    eye4 = np.eye(4, dtype=np.float32)

    shared = {
        "d1t": d1t, "m1t": m1t, "d2t": d2t, "m2t": m2t,
        "s3t": s3t, "s3sqt": s3sqt,
        "x0l0t": np.ascontiguousarray(X0[l0].T).astype(NPBF),
        "l1i": _idx16(l1), "l2i": _idx16(l2),
        "epsv": np.full((128, 1), EPS, np.float32),
        "identbf": np.eye(128, dtype=np.float32).astype(NPBF),
    }
    per_core = []
    for k in range(NCORES):
        m = dict(shared)
        m["xgsel"] = xgsel[k]
        for t in range(3):
            bw = np.zeros((96, 128), np.float32)
            for j in range(4):
                bg = 4 * k + j
                bw[3 * bg:3 * bg + 3, 32 * j:32 * j + 32] = Wm0[t]
            m[f"bigw0_{t}"] = bw.astype(NPBF)
        for lev in range(4):
            for t in range(3):
                m[f"bigw{lev + 1}_{t}"] = np.kron(eye4, Wms[lev][t]).astype(NPBF)
        for lev, nm in ((1, "b1"), (2, "b2"), (3, "b3"), (4, "b4"), (5, "b5")):
            m[f"bias{lev}"] = np.tile(d[nm].astype(np.float32), 4).reshape(128, 1)
        for li in (6, 7, 8):
            W = d[f"W{li}"].astype(np.float32)[:, 512 * k:512 * k + 512]
            m[f"w{li}"] = _tile_w(W, 8).astype(NPBF)
            m[f"g{li}"] = np.ascontiguousarray(
                d[f"g{li}"].astype(np.float32)[512 * k:512 * k + 512].reshape(4, 128).T)
            m[f"be{li}"] = np.ascontiguousarray(
                d[f"be{li}"].astype(np.float32)[512 * k:512 * k + 512].reshape(4, 128).T)
        m["w9"] = _tile_w(
            d["W9"].astype(np.float32)[512 * k:512 * k + 512], 4).astype(NPBF)
        per_core.append(m)

    meta = {"ncw_ops": ncw_ops, "win_of_chunk": win_of_chunk}
    return per_core, meta


# ---------------------------------------------------------------- device program
def _build_nc(meta, shapes, debug=False):
    nc = bacc.Bacc("TRN2", target_bir_lowering=False, debug=False, num_devices=NCORES)
    ein = {}
    for name, arr in shapes.items():
        dt = {np.dtype(np.int16): I16, np.dtype(NPBF): BF16}.get(arr.dtype, F32)
        ein[name] = nc.dram_tensor(name, list(arr.shape), dt, kind="ExternalInput")
    out_mu = nc.dram_tensor("mu", [128, 32], F32, kind="ExternalOutput")

    tap_loc = nc.dram_tensor("tap_loc", [256, 512], BF16)
    tap_all = nc.dram_tensor("tap_all", [2048, 512], BF16, addr_space="Shared")
    z1n_dram = nc.dram_tensor("z1n_dram", [N1, 128], BF16)
    z2n_dram = nc.dram_tensor("z2n_dram", [N2, 128], BF16)
    x6_loc = nc.dram_tensor("x6_loc", [4096, 4], BF16)
    x6_all = nc.dram_tensor("x6_all", [8 * 4096, 4], BF16, addr_space="Shared")
    h_loc = {li: nc.dram_tensor(f"h_loc{li}", [512, 32], BF16) for li in (6, 7)}
    h_all = {li: nc.dram_tensor(f"h_all{li}", [4096, 32], BF16, addr_space="Shared")
             for li in (6, 7)}
    dbg = {}
    if debug:
        for nm, shp in (("z1T", [128, 4096]), ("tapD1", [128, 1024]),
                        ("tapM1", [128, 1024]), ("t0l1", [128, 1024]),
                        ("z2T", [128, 1024]), ("z3T", [128, 128]),
                        ("z5T", [128, 128]), ("x6T", [128, 1024]),
                        ("h6", [128, 128])):
            dbg[nm] = nc.dram_tensor("dbg_" + nm, shp, BF16, kind="ExternalOutput")
    mu_loc = nc.dram_tensor("mu_loc", [128, 32], F32)
    mu_all = nc.dram_tensor("mu_all", [8 * 128, 32], F32, addr_space="Shared")

    ncw_ops = meta["ncw_ops"]
    woc = meta["win_of_chunk"]
    ctot = len(woc)
    c_op0 = int(sum(ncw_ops[0]))
    # first/last-chunk flags per (op, window) group
    first, last = [False] * ctot, [False] * ctot
    prev = None
    for c in range(ctot):
        key = (c < c_op0, woc[c])
        if key != prev:
            first[c] = True
            if c > 0:
                last[c - 1] = True
            prev = key
    last[ctot - 1] = True

    GRP = 16

    with tile.TileContext(nc) as tc:
        with (
            tc.tile_pool(name="const", bufs=1) as cpool,
            tc.tile_pool(name="big", bufs=1) as bigpool,
            tc.tile_pool(name="work", bufs=3) as wpool,
            tc.tile_pool(name="stream", bufs=2) as spool,
            tc.tile_pool(name="wload", bufs=2) as wlpool,
            tc.tile_pool(name="mw", bufs=1) as mwpool,
            tc.tile_pool(name="psA", bufs=2, space="PSUM") as ppool,
            tc.tile_pool(name="psT", bufs=2, space="PSUM") as tpool,
            tc.tile_pool(name="psB", bufs=1, space="PSUM") as apool,
        ):
            def load_const(name, dtype=BF16):
                t = cpool.tile(list(shapes[name].shape), dtype, tag=name, name=name)
                nc.sync.dma_start(out=t[:], in_=ein[name][:, :])
                return t

            eps_t = load_const("epsv", F32)
            identbf = load_const("identbf")

            def transp(src_ap, dst_ap):
                p, f = src_ap.shape
                ps = tpool.tile([128, 128], BF16, tag="pst", name="pst")
                nc.tensor.transpose(out=ps[:f, :p], in_=src_ap, identity=identbf[:p, :p])
                nc.scalar.activation(out=dst_ap, in_=ps[:f, :p], func=AF.Copy)

            # ================= LEVEL 0: streamed selection matmuls ========
            with nc.named_scope("l0_stream"):
                cur = {}
                st = None
                for c in range(ctot):
                    if c % GRP == 0:
                        take = min(GRP, ctot - c)
                        st = spool.tile([128, GRP * 256], BF16, tag="xgsel", name="xgsel")
                        nc.sync.dma_start(
                            out=st[:, :take * 256],
                            in_=ein["xgsel"][:, c * 256:(c + take) * 256])
                    opi = 0 if c < c_op0 else 1
                    w = woc[c]
                    if first[c]:
                        cur[(opi, w)] = ppool.tile([128, 512], F32, tag="ps", name="ps")
                    lo = (c % GRP) * 256
                    nc.tensor.matmul(
                        out=cur[(opi, w)][:, :128],
                        lhsT=st[:, lo:lo + 128], rhs=st[:, lo + 128:lo + 256],
                        start=first[c], stop=last[c])
                    if last[c]:
                        ev = wpool.tile([96, 128], BF16, tag="ev0", name="ev0", bufs=4)
                        nc.scalar.activation(out=ev[:], in_=cur[(opi, w)][:96, :128],
                                             func=AF.Copy)
                        nc.sync.dma_start(
                            out=tap_loc[opi * 128:opi * 128 + 96, w * 128:(w + 1) * 128],
                            in_=ev[:])
                        del cur[(opi, w)]
            with nc.named_scope("ag_taps"):
                nc.gpsimd.collective_compute(
                    "AllGather", ALU.bypass, replica_groups=RG,
                    ins=[tap_loc.ap().opt()], outs=[tap_all.ap().opt()])

            # ================= LEVEL 0 einsum -> z1T ======================
            z1T = bigpool.tile([128, 4096], BF16, tag="z1T", name="z1T")
            z1n = bigpool.tile([128, 4096], BF16, tag="z1n", name="z1n")
            with nc.named_scope("l0_einsum"):
                bw0 = [load_const(f"bigw0_{t}") for t in range(3)]
                bias1 = load_const("bias1", F32)
                for j in range(8):
                    taps = []
                    t0 = wpool.tile([96, 512], BF16, tag="t0", name="t0")
                    nc.sync.dma_start(out=t0[:], in_=ein["x0l0t"][:, 512 * j:512 * (j + 1)])
                    taps.append(t0)
                    for oi in range(2):
                        tt = wpool.tile([96, 512], BF16, tag=f"t{oi + 1}", name=f"t{oi + 1}")
                        r0_ = 256 * j + 128 * oi
                        nc.sync.dma_start(out=tt[:], in_=tap_all[r0_:r0_ + 96, :])
                        taps.append(tt)
                    ps = ppool.tile([128, 512], F32, tag="ps", name="ps")
                    for t in range(3):
                        nc.tensor.matmul(out=ps[:, :], lhsT=bw0[t][:, :], rhs=taps[t][:],
                                         start=(t == 0), stop=(t == 2))
                    nc.scalar.activation(out=z1T[:, 512 * j:512 * (j + 1)], in_=ps[:, :],
                                         func=AF.Identity, bias=bias1[:, 0:1])
                for t in range(32):
                    transp(z1T[:, 128 * t:128 * (t + 1)], z1n[:, 128 * t:128 * (t + 1)])
                nc.sync.dma_start(
                    out=z1n_dram.ap().rearrange("(t p) f -> p t f", p=128),
                    in_=z1n[:].rearrange("p (t f) -> p t f", f=128))

            # ================= LEVEL 1: dense taps ========================
            if debug:
                nc.sync.dma_start(out=dbg["z1T"][:, :], in_=z1T[:])
            tapD1 = bigpool.tile([128, 1024], BF16, tag="tapD1", name="tapD1")
            tapM1 = bigpool.tile([128, 1024], BF16, tag="tapM1", name="tapM1")
            t0l1 = bigpool.tile([128, 1024], BF16, tag="t0l1", name="t0l1")
            with nc.named_scope("l1_taps"):
                for opi, (nm, tap) in enumerate((("d1t", tapD1), ("m1t", tapM1))):
                    ps = apool.tile([128, 1024], F32, tag="psL1", name="psL1")
                    for ld in range(8):
                        stw = wlpool.tile([128, 4096], BF16, tag="wld", name="wld")
                        nc.sync.dma_start(out=stw[:], in_=ein[nm][:, 4096 * ld:4096 * (ld + 1)])
                        for tt in range(4):
                            t = 4 * ld + tt
                            for hh in range(2):
                                nc.tensor.matmul(
                                    out=ps[:, 512 * hh:512 * (hh + 1)],
                                    lhsT=z1n[:, 128 * t:128 * (t + 1)],
                                    rhs=stw[:, 1024 * tt + 512 * hh:
                                            1024 * tt + 512 * (hh + 1)],
                                    start=(t == 0), stop=(t == 31),
                                    skip_group_check=True)
                    nc.scalar.activation(out=tap[:], in_=ps[:, :], func=AF.Copy)
                l1i = load_const("l1i", I16)
                nc.gpsimd.dma_gather(
                    out_ap=t0l1[:].rearrange("p (o n) -> p o n", o=1),
                    in_ap=z1n_dram[:, :], idxs_ap=l1i[:, :],
                    num_idxs=1024, num_idxs_reg=1024, elem_size=128,
                    transpose=True, single_packet=False)

            if debug:
                nc.sync.dma_start(out=dbg["tapD1"][:, :], in_=tapD1[:])
                nc.sync.dma_start(out=dbg["tapM1"][:, :], in_=tapM1[:])
                nc.sync.dma_start(out=dbg["t0l1"][:, :], in_=t0l1[:])
            z2T = bigpool.tile([128, 1024], BF16, tag="z2T", name="z2T")
            z2n = bigpool.tile([128, 1024], BF16, tag="z2n", name="z2n")
            with nc.named_scope("l1_einsum"):
                bw1 = [load_const(f"bigw1_{t}") for t in range(3)]
                bias2 = load_const("bias2", F32)
                for w in range(2):
                    ps = ppool.tile([128, 512], F32, tag="ps", name="ps")
                    for t, tap in enumerate((t0l1, tapD1, tapM1)):
                        nc.tensor.matmul(out=ps[:, :], lhsT=bw1[t][:, :],
                                         rhs=tap[:, 512 * w:512 * (w + 1)],
                                         start=(t == 0), stop=(t == 2))
                    nc.scalar.activation(out=z2T[:, 512 * w:512 * (w + 1)], in_=ps[:, :],
                                         func=AF.Tanh, bias=bias2[:, 0:1])
                for t in range(8):
                    transp(z2T[:, 128 * t:128 * (t + 1)], z2n[:, 128 * t:128 * (t + 1)])
                nc.sync.dma_start(
                    out=z2n_dram.ap().rearrange("(t p) f -> p t f", p=128),
                    in_=z2n[:].rearrange("p (t f) -> p t f", f=128))

            if debug:
                nc.sync.dma_start(out=dbg["z2T"][:, :], in_=z2T[:])
            # ================= LEVEL 2 ====================================
            with nc.named_scope("l2"):
                d2c = load_const("d2t")
                m2c = load_const("m2t")
                taps2 = []
                t0l2 = wpool.tile([128, 128], BF16, tag="t0l2", name="t0l2")
                l2i = load_const("l2i", I16)
                nc.gpsimd.dma_gather(
                    out_ap=t0l2[:].rearrange("p (o n) -> p o n", o=1),
                    in_ap=z2n_dram[:, :], idxs_ap=l2i[:, :],
                    num_idxs=128, num_idxs_reg=128, elem_size=128,
                    transpose=True, single_packet=False)
                taps2.append(t0l2)
                for opi, opc in enumerate((d2c, m2c)):
                    ps = ppool.tile([128, 512], F32, tag="ps", name="ps")
                    for t in range(8):
                        nc.tensor.matmul(out=ps[:, :128],
                                         lhsT=z2n[:, 128 * t:128 * (t + 1)],
                                         rhs=opc[:, 128 * t:128 * (t + 1)],
                                         start=(t == 0), stop=(t == 7))
                    tp = wpool.tile([128, 128], BF16, tag=f"tap2{opi}", name=f"tap2{opi}")
                    nc.scalar.activation(out=tp[:], in_=ps[:, :128], func=AF.Copy)
                    taps2.append(tp)
                bw2 = [load_const(f"bigw2_{t}") for t in range(3)]
                bias3 = load_const("bias3", F32)
                ps = ppool.tile([128, 512], F32, tag="ps", name="ps")
                for t in range(3):
                    nc.tensor.matmul(out=ps[:, :128], lhsT=bw2[t][:, :],
                                     rhs=taps2[t][:, :], start=(t == 0), stop=(t == 2))
                z3T = wpool.tile([128, 128], BF16, tag="z3T", name="z3T")
                nc.scalar.activation(out=z3T[:], in_=ps[:, :128], func=AF.Tanh,
                                     bias=bias3[:, 0:1])
                if debug:
                    nc.sync.dma_start(out=dbg["z3T"][:, :], in_=z3T[:])
                z3n = wpool.tile([128, 128], BF16, tag="z3n", name="z3n")
                transp(z3T[:], z3n[:])

            # ================= LEVEL 3 ====================================
            with nc.named_scope("l3"):
                s3c = load_const("s3t")
                s3sqc = load_const("s3sqt")

                def conv3(zn, zT, bwp, bias_t, func, kp):
                    taps3 = [zT]
                    for oi, opc in enumerate((s3c, s3sqc)):
                        ps = ppool.tile([128, 512], F32, tag="ps", name="ps")
                        nc.tensor.matmul(out=ps[:, :128], lhsT=zn[:], rhs=opc[:, :],
                                         start=True, stop=True)
                        tp = wpool.tile([128, 128], BF16, tag=f"{kp}t{oi}", name=f"{kp}t{oi}")
                        nc.scalar.activation(out=tp[:], in_=ps[:, :128], func=AF.Copy)
                        taps3.append(tp)
                    bw = [load_const(f"{bwp}_{t}") for t in range(3)]
                    ps = ppool.tile([128, 512], F32, tag="ps", name="ps")
                    for t in range(3):
                        nc.tensor.matmul(out=ps[:, :128], lhsT=bw[t][:, :],
                                         rhs=taps3[t][:, :], start=(t == 0), stop=(t == 2))
                    oT = wpool.tile([128, 128], BF16, tag=f"{kp}oT", name=f"{kp}oT")
                    nc.scalar.activation(out=oT[:], in_=ps[:, :128], func=func,
                                         bias=bias_t[:, 0:1])
                    on = wpool.tile([128, 128], BF16, tag=f"{kp}on", name=f"{kp}on")
                    transp(oT[:], on[:])
                    return oT, on

                bias4 = load_const("bias4", F32)
                bias5 = load_const("bias5", F32)
                z4T, z4n = conv3(z3n[:], z3T[:], "bigw3", bias4, AF.Tanh, "c4")
                z5T, _ = conv3(z4n[:], z4T[:], "bigw4", bias5, AF.Identity, "c5")

            if debug:
                nc.sync.dma_start(out=dbg["z5T"][:, :], in_=z5T[:])
            # ================= MLP input assembly =========================
            with nc.named_scope("mlp_in"):
                z5n = wpool.tile([128, 128], BF16, tag="z5n", name="z5n")
                transp(z5T[:], z5n[:])
                for b in range(4):
                    nc.sync.dma_start(
                        out=x6_loc.ap()[:, b:b + 1].rearrange("(n h) o -> n (h o)", h=32),
                        in_=z5n[:, 32 * b:32 * b + 32])
                nc.gpsimd.collective_compute(
                    "AllGather", ALU.bypass, replica_groups=RG,
                    ins=[x6_loc.ap().opt()], outs=[x6_all.ap().opt()])

            # ================= MLP ========================================
            def mlp_layer(nm, src_sb, out_sb):
                g_t = load_const("g" + nm[1], F32)
                be_t = load_const("be" + nm[1], F32)
                wts = []
                for i in range(4):
                    wt = mwpool.tile([128, 4096], BF16, tag=f"mw{i}", name=f"mw{i}")
                    nc.sync.dma_start(out=wt[:], in_=ein[nm][128 * i:128 * (i + 1), :])
                    wts.append(wt)
                for mm in range(4):
                    acc = apool.tile([128, 32], F32, tag="accq", name="accq", bufs=2)
                    for kc in range(32):
                        i, a = kc // 8, kc % 8
                        nc.tensor.matmul(
                            out=acc[:],
                            lhsT=wts[i][:, a * 512 + mm * 128: a * 512 + mm * 128 + 128],
                            rhs=src_sb[:, 32 * kc:32 * kc + 32],
                            start=(kc == 0), stop=(kc == 31))
                    t = wpool.tile([128, 32], F32, tag="b_t", name="b_t")
                    nc.vector.tensor_copy(t[:], acc[:])
                    s1 = wpool.tile([128, 1], F32, tag="b_s1", name="b_s1")
                    nc.vector.tensor_reduce(out=s1[:], in_=t[:], axis=AX.X, op=ALU.add)
                    mu_ = wpool.tile([128, 1], F32, tag="b_mu", name="b_mu")
                    nc.vector.tensor_scalar_mul(mu_[:], s1[:], 1.0 / 32.0)
                    sq = wpool.tile([128, 32], F32, tag="b_sq", name="b_sq")
                    nc.vector.tensor_mul(sq[:], t[:], t[:])
                    s2_ = wpool.tile([128, 1], F32, tag="b_s2", name="b_s2")
                    nc.vector.tensor_reduce(out=s2_[:], in_=sq[:], axis=AX.X, op=ALU.add)
                    var = wpool.tile([128, 1], F32, tag="b_var", name="b_var")
                    nc.vector.scalar_tensor_tensor(out=var[:], in0=mu_[:], scalar=-1.0,
                                                   in1=mu_[:], op0=ALU.mult, op1=ALU.mult)
                    nc.vector.scalar_tensor_tensor(out=var[:], in0=s2_[:], scalar=1.0 / 32.0,
                                                   in1=var[:], op0=ALU.mult, op1=ALU.add)
                    sd = wpool.tile([128, 1], F32, tag="b_sd", name="b_sd")
                    nc.scalar.activation(out=sd[:], in_=var[:], func=AF.Sqrt,
                                         bias=eps_t[:, 0:1])
                    rs = wpool.tile([128, 1], F32, tag="b_rs", name="b_rs")
                    nc.vector.reciprocal(rs[:], sd[:])
                    a_ = wpool.tile([128, 1], F32, tag="b_a", name="b_a")
                    nc.vector.tensor_mul(a_[:], rs[:], g_t[:, mm:mm + 1])
                    sh = wpool.tile([128, 1], F32, tag="b_sh", name="b_sh")
                    nc.vector.scalar_tensor_tensor(out=sh[:], in0=mu_[:], scalar=-1.0,
                                                   in1=a_[:], op0=ALU.mult, op1=ALU.mult)
                    nc.vector.tensor_add(sh[:], sh[:], be_t[:, mm:mm + 1])
                    nc.scalar.activation(out=out_sb[:, 32 * mm:32 * mm + 32], in_=t[:],
                                         func=AF.Relu, scale=a_[:, 0:1], bias=sh[:, 0:1])

            x6T = bigpool.tile([128, 1024], BF16, tag="x6T", name="x6T")
            with nc.named_scope("mlp6"):
                for j in range(8):
                    nc.sync.dma_start(
                        out=x6T[:].rearrange("p (t b) -> p t b", b=32)[:, :, 4 * j:4 * j + 4],
                        in_=x6_all[4096 * j:4096 * (j + 1), :].rearrange(
                            "(t p) b -> p t b", p=128))
                if debug:
                    nc.sync.dma_start(out=dbg["x6T"][:, :], in_=x6T[:])
                h6 = bigpool.tile([128, 128], BF16, tag="h6", name="h6")
                mlp_layer("w6", x6T, h6)
                if debug:
                    nc.sync.dma_start(out=dbg["h6"][:, :], in_=h6[:])
                nc.sync.dma_start(out=h_loc[6].ap().rearrange("(m p) b -> p m b", p=128),
                                  in_=h6[:].rearrange("p (m b) -> p m b", b=32))
                nc.gpsimd.collective_compute(
                    "AllGather", ALU.bypass, replica_groups=RG,
                    ins=[h_loc[6].ap().opt()], outs=[h_all[6].ap().opt()])
            with nc.named_scope("mlp7"):
                x7T = bigpool.tile([128, 1024], BF16, tag="x7T", name="x7T")
                nc.sync.dma_start(out=x7T[:].rearrange("p (t b) -> p t b", b=32),
                                  in_=h_all[6][:, :].rearrange("(t p) b -> p t b", p=128))
                h7 = bigpool.tile([128, 128], BF16, tag="h7", name="h7")
                mlp_layer("w7", x7T, h7)
                nc.sync.dma_start(out=h_loc[7].ap().rearrange("(m p) b -> p m b", p=128),
                                  in_=h7[:].rearrange("p (m b) -> p m b", b=32))
                nc.gpsimd.collective_compute(
                    "AllGather", ALU.bypass, replica_groups=RG,
                    ins=[h_loc[7].ap().opt()], outs=[h_all[7].ap().opt()])
            with nc.named_scope("mlp8"):
                x8T = bigpool.tile([128, 1024], BF16, tag="x8T", name="x8T")
                nc.sync.dma_start(out=x8T[:].rearrange("p (t b) -> p t b", b=32),
                                  in_=h_all[7][:, :].rearrange("(t p) b -> p t b", p=128))
                h8 = bigpool.tile([128, 128], BF16, tag="h8", name="h8")
                mlp_layer("w8", x8T, h8)

            with nc.named_scope("mlp9"):
                w9t = load_const("w9")
                ps9 = apool.tile([128, 32], F32, tag="accq", name="accq", bufs=2)
                for kc in range(4):
                    nc.tensor.matmul(out=ps9[:], lhsT=w9t[:, kc * 128:(kc + 1) * 128],
                                     rhs=h8[:, 32 * kc:32 * kc + 32],
                                     start=(kc == 0), stop=(kc == 3))
                mu_sb = wpool.tile([128, 32], F32, tag="mu_sb", name="mu_sb")
                nc.scalar.activation(out=mu_sb[:], in_=ps9[:], func=AF.Copy)
                nc.sync.dma_start(out=mu_loc[:, :], in_=mu_sb[:])
                nc.gpsimd.collective_compute(
                    "AllGather", ALU.bypass, replica_groups=RG,
                    ins=[mu_loc.ap().opt()], outs=[mu_all.ap().opt()])
                tot = wpool.tile([128, 32], F32, tag="f_tot", name="f_tot")
                nc.sync.dma_start(out=tot[:], in_=mu_all[0:128, :])
                for k in range(1, 8):
                    pk = wpool.tile([128, 32], F32, tag="f_pk", name="f_pk")
                    nc.sync.dma_start(out=pk[:], in_=mu_all[k * 128:(k + 1) * 128, :])
                    nc.vector.tensor_add(tot[:], tot[:], pk[:])
                s1 = wpool.tile([128, 1], F32, tag="f_s1", name="f_s1")
                nc.vector.tensor_reduce(out=s1[:], in_=tot[:], axis=AX.X, op=ALU.add)
                mu_ = wpool.tile([128, 1], F32, tag="f_mu", name="f_mu")
                nc.vector.tensor_scalar_mul(mu_[:], s1[:], 1.0 / 32.0)
                sq = wpool.tile([128, 32], F32, tag="f_sq", name="f_sq")
                nc.vector.tensor_mul(sq[:], tot[:], tot[:])
                s2_ = wpool.tile([128, 1], F32, tag="f_s2", name="f_s2")
                nc.vector.tensor_reduce(out=s2_[:], in_=sq[:], axis=AX.X, op=ALU.add)
                var = wpool.tile([128, 1], F32, tag="f_var", name="f_var")
                nc.vector.scalar_tensor_tensor(out=var[:], in0=mu_[:], scalar=-1.0,
                                               in1=mu_[:], op0=ALU.mult, op1=ALU.mult)
                nc.vector.scalar_tensor_tensor(out=var[:], in0=s2_[:], scalar=1.0 / 32.0,
                                               in1=var[:], op0=ALU.mult, op1=ALU.add)
                sdf = wpool.tile([128, 1], F32, tag="f_sd", name="f_sd")
                nc.scalar.activation(out=sdf[:], in_=var[:], func=AF.Sqrt,
                                     bias=eps_t[:, 0:1])
                rs = wpool.tile([128, 1], F32, tag="f_rs", name="f_rs")
                nc.vector.reciprocal(rs[:], sdf[:])
                neg = wpool.tile([128, 1], F32, tag="f_neg", name="f_neg")
                nc.vector.scalar_tensor_tensor(out=neg[:], in0=mu_[:], scalar=-1.0,
                                               in1=rs[:], op0=ALU.mult, op1=ALU.mult)
                outt = wpool.tile([128, 32], F32, tag="f_out", name="f_out")
                nc.scalar.activation(out=outt[:], in_=tot[:], func=AF.Identity,
                                     scale=rs[:, 0:1], bias=neg[:, 0:1])
                nc.sync.dma_start(out=out_mu[:, :], in_=outt[:])

    nc.compile()
    return nc


# ---------------------------------------------------------------- entry point
def kernel(**inputs) -> np.ndarray:
    per_core, meta = _host_prep(inputs)
    key = (len(meta["win_of_chunk"]), tuple(meta["win_of_chunk"]),
           tuple(tuple(n) for n in meta["ncw_ops"]))
    if _CACHE.get("key") != key:
        _CACHE["prog"] = _build_nc(meta, per_core[0])
        _CACHE["key"] = key
    nc = _CACHE["prog"]
    res = bass_utils.run_bass_kernel_spmd(nc, per_core, core_ids=list(range(NCORES)))
    return np.ascontiguousarray(res.results[0]["mu"].T)
